# revision 1
# baseline (speedup 1.0000x reference)
"""SchNet forward on 8 Trainium2 NeuronCores (Bass/Tile), data-parallel over molecules.

kernel(**inputs) takes FULL inputs (as produced by setup_inputs) and returns
the FULL [256] float32 per-molecule energies. Inside: shards 256 molecules
into 8 groups of 32 (1024 atoms each), runs an SPMD Bass kernel on cores 0-7,
gathers outputs.

Hardcoded shape: N=8192 atoms, 32 atoms/molecule, FEAT=100, NG=25, K=28, L=4,
CUTOFF=6.  Per core: 1024 atoms, all-pairs 32x32 block distances (E=32768
edge slots); top-28 selection done by rank counting; non-selected edges get
distance=CUTOFF so the cosine cutoff zeroes them exactly like the reference's
top_k + ccut weighting.
"""

import math
import numpy as np

N = 8192
APM = 32
FEAT = 100
NG = 25
K = 28
L = 4
CUTOFF = 6.0
NCORES = 8
NA = N // NCORES          # atoms per core = 1024
NM = NA // APM            # molecules per core = 32
E = NA * APM              # edge slots per core = 32768
EG = E // 4               # edges per partition-group = 8192
EC = 1024                 # edge chunk = one molecule's 32x32 pairs
H = FEAT // 2
NBLK = NA // 128          # 8 atom blocks per core

_COMPILED = None


def _build(repeats: int = 1):
    import concourse.bass as bass
    import concourse.mybir as mybir
    import concourse.tile as tile
    from concourse import bacc

    dt = mybir.dt
    F32 = dt.float32
    F32R = dt.float32r
    A = mybir.ActivationFunctionType
    OP = mybir.AluOpType
    AX = mybir.AxisListType
    LF = L * FEAT

    nc = bacc.Bacc()

    pos_d = nc.dram_tensor("pos", [NA, 3], F32, kind="ExternalInput")
    h0_d = nc.dram_tensor("h0", [FEAT, NA], F32, kind="ExternalInput")
    w1rep_d = nc.dram_tensor("w1rep", [L, 128, FEAT], F32, kind="ExternalInput")
    w2_d = nc.dram_tensor("w2", [L, FEAT, FEAT], F32, kind="ExternalInput")
    b1_d = nc.dram_tensor("b1", [L, FEAT], F32, kind="ExternalInput")
    b2_d = nc.dram_tensor("b2", [L, FEAT], F32, kind="ExternalInput")
    l1w_d = nc.dram_tensor("l1w", [L, FEAT, FEAT], F32, kind="ExternalInput")
    l2w_d = nc.dram_tensor("l2w", [L, FEAT, FEAT], F32, kind="ExternalInput")
    l2b_d = nc.dram_tensor("l2b", [L, FEAT], F32, kind="ExternalInput")
    lw_d = nc.dram_tensor("lw", [L, FEAT, FEAT], F32, kind="ExternalInput")
    lb_d = nc.dram_tensor("lb", [L, FEAT], F32, kind="ExternalInput")
    ow1_d = nc.dram_tensor("ow1", [FEAT, H], F32, kind="ExternalInput")
    ob1_d = nc.dram_tensor("ob1", [H], F32, kind="ExternalInput")
    ow2_d = nc.dram_tensor("ow2", [H, 1], F32, kind="ExternalInput")
    ob2_d = nc.dram_tensor("ob2", [1], F32, kind="ExternalInput")
    diag_d = nc.dram_tensor("diagc", [128, APM], F32, kind="ExternalInput")
    offs_d = nc.dram_tensor("offs", [128, 1], F32, kind="ExternalInput")

    out_d = nc.dram_tensor("energy", [NM], F32, kind="ExternalOutput")

    dtil_dram = nc.dram_tensor("dtil_lin", [E], F32)
    gam_dram = nc.dram_tensor("gam_lin", [E], F32R)

    def bap(a, off, dims):
        return bass.AP(tensor=a.tensor, offset=a.offset + off, ap=dims)

    with tile.TileContext(nc) as tc:
        import contextlib
        ctx = contextlib.ExitStack()
        with ctx:
            persist = ctx.enter_context(tc.tile_pool(name="persist", bufs=1))
            wpool = ctx.enter_context(tc.tile_pool(name="weights", bufs=1))
            psA = ctx.enter_context(tc.tile_pool(name="psA", bufs=2, space="PSUM"))
            psB = ctx.enter_context(tc.tile_pool(name="psB", bufs=2, space="PSUM"))

            # persistent tiles
            ea0 = persist.tile([128, EG], F32R, tag="ea0")   # groups 0(base0),1(base64)
            ea1 = persist.tile([128, EG], F32R, tag="ea1")   # groups 2(base0),3(base64)
            hA = persist.tile([FEAT, NA], F32, tag="hA")
            hB = persist.tile([FEAT, NA], F32, tag="hB")
            x1_t = persist.tile([FEAT, NA], F32, tag="x1")
            agg_t = persist.tile([FEAT, NA], F32, tag="agg")
            half_t = persist.tile([128, 1], F32, tag="half")
            nhalfpi_t = persist.tile([128, 1], F32, tag="nhalfpi")
            diag_t = persist.tile([128, APM], F32, tag="diag")
            offs_t = persist.tile([128, 1], F32, tag="offs")
            nc.vector.memset(half_t[:], 0.5)
            nc.vector.memset(nhalfpi_t[:], -math.pi / 2)
            nc.sync.dma_start(out=diag_t[:], in_=diag_d[:])
            nc.sync.dma_start(out=offs_t[:], in_=offs_d[:])

            # weights
            w1f = wpool.tile([128, LF], F32, tag="w1f")
            w1_t = wpool.tile([128, LF], F32R, tag="w1")
            w2f = wpool.tile([FEAT, LF], F32, tag="w2f")
            w2_t = wpool.tile([FEAT, LF], F32R, tag="w2")
            b2f = wpool.tile([128, LF], F32, tag="b2f")
            b2r_t = wpool.tile([128, LF], F32R, tag="b2r")   # row 64 holds b2 per layer
            l1w_t = wpool.tile([FEAT, LF], F32, tag="l1w")
            l2w_t = wpool.tile([FEAT, LF], F32, tag="l2w")
            lw_t = wpool.tile([FEAT, LF], F32, tag="lww")
            b1_t = wpool.tile([FEAT, L], F32, tag="b1")
            l2b_t = wpool.tile([FEAT, L], F32, tag="l2b")
            lb_t = wpool.tile([FEAT, L], F32, tag="lb")
            ow1_t = wpool.tile([FEAT, H], F32, tag="ow1")
            ob1_t = wpool.tile([H, 1], F32, tag="ob1")
            ow2_t = wpool.tile([H, 1], F32, tag="ow2")
            ob2_t = wpool.tile([1, 1], F32, tag="ob2")

            nc.sync.dma_start(out=w1f[:].rearrange("p (l f) -> p l f", f=FEAT),
                              in_=w1rep_d[:].transpose([1, 0, 2]))
            nc.vector.tensor_copy(w1_t[:], w1f[:])
            nc.sync.dma_start(out=w2f[:].rearrange("p (l f) -> p l f", f=FEAT),
                              in_=w2_d[:].transpose([1, 0, 2]))
            nc.vector.tensor_copy(w2_t[:], w2f[:])
            nc.vector.memset(b2f[:], 0.0)
            nc.sync.dma_start(
                out=b2f[64:65, :].rearrange("p (l f) -> p l f", f=FEAT),
                in_=b2_d[:].unsqueeze(0))
            nc.vector.tensor_copy(b2r_t[:], b2f[:])
            nc.sync.dma_start(out=l1w_t[:].rearrange("p (l f) -> p l f", f=FEAT),
                              in_=l1w_d[:].transpose([1, 0, 2]))
            nc.sync.dma_start(out=l2w_t[:].rearrange("p (l f) -> p l f", f=FEAT),
                              in_=l2w_d[:].transpose([1, 0, 2]))
            nc.sync.dma_start(out=lw_t[:].rearrange("p (l f) -> p l f", f=FEAT),
                              in_=lw_d[:].transpose([1, 0, 2]))
            nc.sync.dma_start(out=b1_t[:], in_=b1_d[:].transpose([1, 0]))
            nc.sync.dma_start(out=l2b_t[:], in_=l2b_d[:].transpose([1, 0]))
            nc.sync.dma_start(out=lb_t[:], in_=lb_d[:].transpose([1, 0]))
            nc.sync.dma_start(out=ow1_t[:], in_=ow1_d[:])
            nc.sync.dma_start(out=ob1_t[:], in_=ob1_d[:].unsqueeze(1))
            nc.sync.dma_start(out=ow2_t[:], in_=ow2_d[:])
            nc.sync.dma_start(out=ob2_t[:], in_=ob2_d[:].unsqueeze(1))

            for rep in range(repeats):
                nc.sync.dma_start(out=hA[:], in_=h0_d[:])
                pA = tc.tile_pool(name=f"bld{rep}", bufs=1)
                pAs = tc.tile_pool(name=f"scrA{rep}", bufs=2)
                with pA as bp, pAs as sc:
                    # ========== PHASE A: graph build ==========
                    EA_ = NBLK * APM
                    d2all = bp.tile([128, EA_], F32, tag="d2all")
                    for b in range(NBLK):
                        posP = sc.tile([128, 3], F32, tag="posP")
                        nc.sync.dma_start(out=posP[:], in_=pos_d[128 * b:128 * (b + 1), :])
                        posB = sc.tile([128, APM, 3], F32, tag="posB")
                        nc.sync.dma_start(
                            out=posB[:],
                            in_=bap(pos_d[:], 4 * b * APM * 3,
                                    [[APM * 3, 4], [0, APM], [3, APM], [1, 3]]))
                        dif = sc.tile([128, APM, 3], F32, tag="dif")
                        pP = posP[:]
                        nc.vector.tensor_tensor(
                            out=dif[:],
                            in0=bap(pP, 0, [pP.ap[0], [0, APM], [1, 3]]),
                            in1=posB[:], op=OP.subtract)
                        sq = sc.tile([128, APM, 3], F32, tag="sq")
                        nc.vector.tensor_tensor(out=sq[:], in0=dif[:], in1=dif[:],
                                                op=OP.mult)
                        nc.vector.tensor_reduce(out=d2all[:, APM * b:APM * (b + 1)],
                                                in_=sq[:], axis=AX.X, op=OP.add)
                    gtm = bp.tile([128, EA_], F32, tag="gtm")
                    nc.vector.tensor_scalar(out=gtm[:], in0=d2all[:], scalar1=36.0,
                                            scalar2=None, op0=OP.is_gt)
                    mask = bp.tile([128, EA_], F32, tag="mask")
                    for b in range(NBLK):
                        nc.vector.tensor_tensor(out=mask[:, APM * b:APM * (b + 1)],
                                                in0=gtm[:, APM * b:APM * (b + 1)],
                                                in1=diag_t[:], op=OP.max)
                    inv = bp.tile([128, EA_], F32, tag="inv")
                    nc.vector.tensor_scalar(out=inv[:], in0=mask[:], scalar1=-1.0,
                                            scalar2=1.0, op0=OP.mult, op1=OP.add)
                    d2m = bp.tile([128, EA_], F32, tag="d2m")
                    nc.vector.tensor_tensor(out=d2m[:], in0=d2all[:], in1=inv[:],
                                            op=OP.mult)
                    m36 = bp.tile([128, EA_], F32, tag="m36")
                    nc.vector.tensor_scalar(out=m36[:], in0=mask[:], scalar1=36.0,
                                            scalar2=None, op0=OP.mult)
                    nc.vector.tensor_tensor(out=d2m[:], in0=d2m[:], in1=m36[:],
                                            op=OP.add)

                    sel = bp.tile([128, EA_], F32, tag="sel")
                    for b in range(NBLK):
                        dd = d2m[:, APM * b:APM * (b + 1)]
                        lt = sc.tile([128, APM, APM], F32, tag="lt")
                        nc.vector.tensor_tensor(
                            out=lt[:],
                            in0=bap(dd, 0, [dd.ap[0], [0, APM], [1, APM]]),
                            in1=bap(dd, 0, [dd.ap[0], [1, APM], [0, APM]]),
                            op=OP.is_lt)
                        rk = sc.tile([128, APM], F32, tag="rk")
                        nc.vector.tensor_reduce(out=rk[:], in_=lt[:], axis=AX.X,
                                                op=OP.add)
                        nc.vector.tensor_scalar(out=sel[:, APM * b:APM * (b + 1)],
                                                in0=rk[:], scalar1=float(K) - 0.5,
                                                scalar2=None, op0=OP.is_lt)

                    s_t = bp.tile([128, EA_], F32, tag="s_t")
                    nc.scalar.activation(s_t[:], d2m[:], A.Sqrt)
                    for _ in range(2):
                        rc = sc.tile([128, EA_], F32, tag="rc")
                        nc.vector.reciprocal(rc[:], s_t[:])
                        tq = sc.tile([128, EA_], F32, tag="tq")
                        nc.vector.tensor_tensor(out=tq[:], in0=d2m[:], in1=rc[:],
                                                op=OP.mult)
                        nc.vector.tensor_tensor(out=s_t[:], in0=s_t[:], in1=tq[:],
                                                op=OP.add)
                        nc.vector.tensor_scalar(out=s_t[:], in0=s_t[:], scalar1=0.5,
                                                scalar2=None, op0=OP.mult)
                    dm6 = bp.tile([128, EA_], F32, tag="dm6")
                    nc.vector.tensor_scalar(out=dm6[:], in0=s_t[:], scalar1=-6.0,
                                            scalar2=None, op0=OP.add)
                    dtil = bp.tile([128, EA_], F32, tag="dtil")
                    nc.vector.tensor_tensor(out=dtil[:], in0=sel[:], in1=dm6[:],
                                            op=OP.mult)
                    nc.vector.tensor_scalar(out=dtil[:], in0=dtil[:], scalar1=6.0,
                                            scalar2=None, op0=OP.add)
                    sn = bp.tile([128, EA_], F32, tag="sn")
                    nc.scalar.activation(sn[:], dtil[:], A.Sin, bias=nhalfpi_t[:],
                                         scale=float(math.pi / 6.0))
                    nc.vector.tensor_scalar(out=sn[:], in0=sn[:], scalar1=-0.5,
                                            scalar2=0.5, op0=OP.mult, op1=OP.add)
                    ilt = bp.tile([128, EA_], F32, tag="ilt")
                    nc.vector.tensor_scalar(out=ilt[:], in0=d2m[:], scalar1=36.0,
                                            scalar2=None, op0=OP.is_lt)
                    nc.vector.tensor_tensor(out=ilt[:], in0=ilt[:], in1=sel[:],
                                            op=OP.mult)
                    gam = bp.tile([128, EA_], F32R, tag="gam")
                    nc.vector.tensor_tensor(out=gam[:], in0=sn[:], in1=ilt[:],
                                            op=OP.mult)

                    for b in range(NBLK):
                        nc.sync.dma_start(
                            out=bap(dtil_dram[:], 4096 * b, [[APM, 128], [1, APM]]),
                            in_=dtil[:, APM * b:APM * (b + 1)])
                        nc.sync.dma_start(
                            out=bap(gam_dram[:], 4096 * b, [[APM, 128], [1, APM]]),
                            in_=gam[:, APM * b:APM * (b + 1)])

                    # drep tiles: tile t holds groups {2t,2t+1} at bases {0,64}
                    for t_i, ea_tile in ((0, ea0), (1, ea1)):
                        drep = bp.tile([128, EG], F32, tag="drep")
                        for gg in range(2):
                            g = 2 * t_i + gg
                            dst = bap(drep[:], 0,
                                      [[drep[:].ap[0][0] * 64, 1],
                                       [drep[:].ap[0][0], 32], [1, EG]])
                            dst = bass.AP(tensor=drep[:].tensor,
                                          offset=drep[:].offset,
                                          ap=[[drep[:].ap[0][0], 32], [1, EG]]) \
                                if gg == 0 else \
                                bass.AP(tensor=drep[:].tensor,
                                        offset=drep[:].offset + 64 * drep[:].ap[0][0],
                                        ap=[[drep[:].ap[0][0], 32], [1, EG]])
                            nc.sync.dma_start(
                                out=dst,
                                in_=bap(dtil_dram[:], EG * g, [[0, 32], [1, EG]]))
                        for cj in range(EG // 2048):
                            ssl = slice(2048 * cj, 2048 * (cj + 1))
                            q = sc.tile([128, 2048], F32, tag="q")
                            nc.vector.tensor_scalar(out=q[:], in0=drep[:, ssl],
                                                    scalar1=offs_t[:], scalar2=None,
                                                    op0=OP.subtract)
                            nc.vector.tensor_tensor(out=q[:], in0=q[:], in1=q[:],
                                                    op=OP.mult)
                            nc.scalar.activation(ea_tile[:, ssl], q[:], A.Exp,
                                                 scale=-8.0)

                with tc.tile_pool(name=f"scrB{rep}", bufs=2) as sc:
                    # ========== PHASE B: interaction layers ==========
                    hcur, hnxt = hA, hB
                    for l in range(L):
                        lf = slice(FEAT * l, FEAT * (l + 1))
                        ps_n = psA.tile([FEAT, NA], F32, tag="psA")
                        for hh in range(2):
                            qs = slice(512 * hh, 512 * (hh + 1))
                            nc.tensor.matmul(ps_n[:, qs], l1w_t[:, lf], hcur[:, qs],
                                             start=True, stop=True)
                        nc.vector.tensor_copy(x1_t[:], ps_n[:])

                        for ci in range(E // EC):
                            g, cj = divmod(ci, EG // EC)
                            ea_tile = ea0 if g < 2 else ea1
                            base = 64 * (g % 2)
                            ps1 = psA.tile([FEAT, EC], F32, tag="psA")
                            for q2 in range(EC // 512):
                                qs = slice(512 * q2, 512 * (q2 + 1))
                                nc.tensor.matmul(
                                    ps1[:, qs],
                                    w1_t[base:base + NG, lf],
                                    ea_tile[base:base + NG,
                                            EC * cj + 512 * q2:EC * cj + 512 * (q2 + 1)],
                                    start=True, stop=True)
                            ue = sc.tile([FEAT, EC], F32, tag="ue")
                            nc.scalar.activation(ue[:], ps1[:], A.Exp,
                                                 bias=b1_t[:, l:l + 1])
                            u = sc.tile([FEAT, EC], F32, tag="u")
                            nc.scalar.activation(u[:], ue[:], A.Ln,
                                                 bias=half_t[:FEAT], scale=0.5)
                            gr = sc.tile([128, EC], F32R, tag="gr")
                            nc.sync.dma_start(
                                out=gr[:],
                                in_=bap(gam_dram[:], EG * g + EC * cj,
                                        [[0, 128], [1, EC]]))
                            up = sc.tile([FEAT, EC], F32R, tag="up")
                            nc.vector.tensor_tensor(out=up[:], in0=u[:],
                                                    in1=gr[:FEAT, :], op=OP.mult)
                            ps2 = psB.tile([FEAT, EC], F32, tag="psB")
                            for q2 in range(EC // 512):
                                qs = slice(512 * q2, 512 * (q2 + 1))
                                nc.tensor.matmul(ps2[:, qs], w2_t[:, lf], up[:, qs],
                                                 start=True, stop=False)
                                nc.tensor.matmul(ps2[:, qs], b2r_t[64:65, lf],
                                                 gr[64:65, qs], start=False, stop=True)
                            a0 = 256 * g + 32 * cj   # first atom of this molecule
                            x1b = x1_t[:]
                            msg = sc.tile([FEAT, EC], F32, tag="msg")
                            nc.vector.tensor_tensor(
                                out=msg[:], in0=ps2[:],
                                in1=bap(x1b, a0, [x1b.ap[0], [0, APM], [1, APM]]),
                                op=OP.mult)
                            nc.vector.tensor_reduce(
                                out=agg_t[:, a0:a0 + APM],
                                in_=msg[:].rearrange("p (a j) -> p a j", j=APM),
                                axis=AX.X, op=OP.add)

                        ps_v = psA.tile([FEAT, NA], F32, tag="psA")
                        for hh in range(2):
                            qs = slice(512 * hh, 512 * (hh + 1))
                            nc.tensor.matmul(ps_v[:, qs], l2w_t[:, lf], agg_t[:, qs],
                                             start=True, stop=True)
                        spe = sc.tile([FEAT, NA], F32, tag="ue")
                        nc.scalar.activation(spe[:], ps_v[:], A.Exp,
                                             bias=l2b_t[:, l:l + 1])
                        spl = sc.tile([FEAT, NA], F32, tag="u")
                        nc.scalar.activation(spl[:], spe[:], A.Ln,
                                             bias=half_t[:FEAT], scale=0.5)
                        ps_w = psB.tile([FEAT, NA], F32, tag="psB")
                        for hh in range(2):
                            qs = slice(512 * hh, 512 * (hh + 1))
                            nc.tensor.matmul(ps_w[:, qs], lw_t[:, lf], spl[:, qs],
                                             start=True, stop=True)
                        nc.vector.scalar_tensor_tensor(
                            out=hnxt[:], in0=ps_w[:], scalar=lb_t[:, l:l + 1],
                            in1=hcur[:], op0=OP.add, op1=OP.add)
                        hcur, hnxt = hnxt, hcur

                    # ========== PHASE C: readout ==========
                    ps_r = psA.tile([FEAT, NA], F32, tag="psA")
                    for hh in range(2):
                        qs = slice(512 * hh, 512 * (hh + 1))
                        nc.tensor.matmul(ps_r[:H, qs], ow1_t[:], hcur[:, qs],
                                         start=True, stop=True)
                    re = sc.tile([H, NA], F32, tag="ue")
                    nc.scalar.activation(re[:], ps_r[:H, :], A.Exp, bias=ob1_t[:])
                    rl = sc.tile([H, NA], F32, tag="u")
                    nc.scalar.activation(rl[:], re[:], A.Ln, bias=half_t[:H],
                                         scale=0.5)
                    ps_e = psB.tile([FEAT, NA], F32, tag="psB")
                    for hh in range(2):
                        qs = slice(512 * hh, 512 * (hh + 1))
                        nc.tensor.matmul(ps_e[:1, qs], ow2_t[:], rl[:, qs],
                                         start=True, stop=True)
                    pa = sc.tile([1, NA], F32, tag="pa")
                    nc.vector.tensor_scalar(out=pa[:], in0=ps_e[:1, :],
                                            scalar1=ob2_t[:1, :], scalar2=None,
                                            op0=OP.add)
                    en = sc.tile([1, NM], F32, tag="en")
                    nc.vector.tensor_reduce(
                        out=en[:], in_=pa[:].rearrange("p (m i) -> p m i", i=APM),
                        axis=AX.X, op=OP.add)
                    nc.sync.dma_start(out=out_d[:].unsqueeze(0), in_=en[:])

    nc.compile()
    return nc


def _prep_inputs(z, pos, ptr, emb, mlp_w1, mlp_b1, mlp_w2, mlp_b2,
                 lin1_w, lin2_w, lin2_b, lin_w, lin_b,
                 out_w1, out_b1, out_w2, out_b2):
    z = np.asarray(z)
    pos = np.ascontiguousarray(np.asarray(pos, dtype=np.float32))
    ptr = np.asarray(ptr)
    assert pos.shape == (N, 3)
    expect = np.arange(0, N + APM, APM)
    assert np.array_equal(ptr.astype(np.int64), expect), "non-uniform molecules unsupported"

    emb = np.asarray(emb, dtype=np.float32)
    w1 = np.asarray(mlp_w1, dtype=np.float32)
    w1rep = np.zeros((L, 128, FEAT), dtype=np.float32)
    for g in range(4):
        w1rep[:, 32 * g:32 * g + NG, :] = w1
    diag = np.zeros((128, APM), dtype=np.float32)
    for p in range(128):
        diag[p, p % APM] = 1.0
    offs = np.zeros((128, 1), dtype=np.float32)
    offvals = np.linspace(0.0, CUTOFF, NG).astype(np.float32)
    for p in range(128):
        if p % 32 < NG:
            offs[p, 0] = offvals[p % 32]

    shared = {
        "w1rep": w1rep,
        "w2": np.ascontiguousarray(mlp_w2, dtype=np.float32),
        "b1": np.ascontiguousarray(mlp_b1, dtype=np.float32),
        "b2": np.ascontiguousarray(mlp_b2, dtype=np.float32),
        "l1w": np.ascontiguousarray(lin1_w, dtype=np.float32),
        "l2w": np.ascontiguousarray(lin2_w, dtype=np.float32),
        "l2b": np.ascontiguousarray(lin2_b, dtype=np.float32),
        "lw": np.ascontiguousarray(lin_w, dtype=np.float32),
        "lb": np.ascontiguousarray(lin_b, dtype=np.float32),
        "ow1": np.ascontiguousarray(out_w1, dtype=np.float32),
        "ob1": np.ascontiguousarray(np.asarray(out_b1, dtype=np.float32)),
        "ow2": np.ascontiguousarray(out_w2, dtype=np.float32),
        "ob2": np.asarray(out_b2, dtype=np.float32).reshape(1),
        "diagc": diag,
        "offs": offs,
    }
    in_maps = []
    for c in range(NCORES):
        sl = slice(NA * c, NA * (c + 1))
        h0 = emb[np.asarray(z[sl], dtype=np.int64)].T
        m = dict(shared)
        m["pos"] = pos[sl].copy()
        m["h0"] = np.ascontiguousarray(h0, dtype=np.float32)
        in_maps.append(m)
    return in_maps


def kernel(**inputs) -> np.ndarray:
    from concourse.bass_utils import run_bass_kernel_spmd
    global _COMPILED
    if _COMPILED is None:
        _COMPILED = _build(1)
    nc = _COMPILED
    in_maps = _prep_inputs(**inputs)
    res = run_bass_kernel_spmd(nc, in_maps, list(range(NCORES)))
    out = np.concatenate([res.results[c]["energy"] for c in range(NCORES)])
    return out.astype(np.float32)


if __name__ == "__main__":
    _build(1)
    print("built ok")



# revision 13
# speedup vs baseline: 27.5004x; 27.5004x over previous
"""SchNet forward on 8 Trainium2 NeuronCores (Bass/Tile), data-parallel over molecules.

kernel(**inputs) takes FULL inputs (as produced by setup_inputs) and returns
the FULL [256] float32 per-molecule energies. Inside: shards 256 molecules
into 8 groups of 32 (1024 atoms each), runs an SPMD Bass kernel on cores 0-7,
gathers outputs.

Per core: 1024 atoms, all-pairs 32x32 block distances (E=32768 edge slots);
top-28 selection by rank counting; non-selected / masked edges get distance
CUTOFF=6 exactly, where the filter is exactly zero.

The edge filter network (2-layer MLP on 25 gaussian features x cosine cutoff)
depends only on the scalar edge distance, so it is evaluated host-side on a
dense grid and least-squares-fitted to a T=64-node piecewise-linear (tent)
basis per layer. On device each edge only needs its tent coefficient vector
c[t,e] = relu(1 - |d/h - t|) (built once, reused by all 4 layers) and one
matmul TBL_l @ c per 512 edges. The tent node at d=6 is pinned to 0 so
masked edges contribute exactly nothing.

ssp(x) = softplus(x) - log(2) is computed with a single Softplus activation;
the -log(2) shift is folded into the following linear layer's bias host-side.

The whole per-repetition body sits inside one hardware For_i loop, so
repeated executions reuse the same static instruction stream.
"""

import math
import numpy as np

N = 8192
APM = 32
FEAT = 100
NG = 25
K = 28
L = 4
CUTOFF = 6.0
NCORES = 8
NA = N // NCORES          # atoms per core = 1024
NM = NA // APM            # molecules per core = 32
E = NA * APM              # edge slots per core = 32768
EH = E // 2               # edges per half (on partition rows 0-63 / 64-127)
T = 64                    # tent-basis nodes over [0, 6]
TH = CUTOFF / (T - 1)
NBLK = NA // 128          # 8 atom blocks per core
CH = 2048                 # edges per chunk (PSUM tile)
NCH = E // CH             # 16 chunks
APC = CH // APM           # atoms per chunk = 64
H = FEAT // 2
LOG2 = float(np.log(2.0))

# wpack column layout
_WC_TBL = 0                      # [128, L*FEAT] tent tables (rows 0:64 and 64:128 identical)
_WC_L1W = _WC_TBL + L * FEAT     # [100, L*FEAT]
_WC_L2W = _WC_L1W + L * FEAT
_WC_LW = _WC_L2W + L * FEAT
_WC_L2B = _WC_LW + L * FEAT      # [100, L]
_WC_LBP = _WC_L2B + L            # [100, L]  lin_b - log2*colsum(lin_w)
_WC_OW1 = _WC_LBP + L            # [100, H]
_WC_OW2 = _WC_OW1 + H            # [H, 1]
_WC_OB1 = _WC_OW2 + 1            # [H, 1]
_WC_OB2 = _WC_OB1 + 1            # [1, 1]  out_b2 - log2*sum(out_w2)
_WC_TV = _WC_OB2 + 1             # [128, 1] tent node index (p % 64)
_WC_D36 = _WC_TV + 1             # [128, APM] 36 at j == p%32
WC = _WC_D36 + APM

_COMPILED = None


def _build(repeats: int = 1):
    import concourse.bass as bass
    import concourse.mybir as mybir
    import concourse.tile as tile
    from concourse import bacc

    dt = mybir.dt
    F32 = dt.float32
    F32R = dt.float32r
    A = mybir.ActivationFunctionType
    OP = mybir.AluOpType
    AX = mybir.AxisListType
    LF = L * FEAT

    nc = bacc.Bacc()

    pos_d = nc.dram_tensor("pos", [NA, 3], F32, kind="ExternalInput")
    h0_d = nc.dram_tensor("h0", [FEAT, NA], F32, kind="ExternalInput")
    wpack_d = nc.dram_tensor("wpack", [128, WC], F32, kind="ExternalInput")
    out_d = nc.dram_tensor("energy", [NM], F32, kind="ExternalOutput")
    dtil_dram = nc.dram_tensor("dtil_lin", [E], F32)

    def bap(a, off, dims):
        return bass.AP(tensor=a.tensor, offset=a.offset + off, ap=dims)

    with tile.TileContext(nc) as tc:
        import contextlib
        ctx = contextlib.ExitStack()
        with ctx:
            wp = ctx.enter_context(tc.tile_pool(name="wp", bufs=1))
            rp = ctx.enter_context(tc.tile_pool(name="rp", bufs=1))
            sc = ctx.enter_context(tc.tile_pool(name="sc", bufs=1))
            psX = ctx.enter_context(tc.tile_pool(name="psX", bufs=1, space="PSUM"))
            psE = ctx.enter_context(tc.tile_pool(name="psE", bufs=1, space="PSUM"))
            psN = ctx.enter_context(tc.tile_pool(name="psN", bufs=1, space="PSUM"))

            w_t = wp.tile([128, WC], F32, tag="wpack")
            nc.sync.dma_start(out=w_t[:], in_=wpack_d[:])
            half_t = wp.tile([128, 1], F32, tag="half")
            nc.vector.memset(half_t[:], 0.5)

            # persistent across reps (recomputed inside each rep)
            h_t = rp.tile([FEAT, NA], F32, tag="h")
            x1s_t = rp.tile([FEAT, NA], F32, tag="x1s")
            agg_t = rp.tile([FEAT, NA], F32, tag="agg")
            c_t = rp.tile([128, EH], F32, tag="tentc")

            wap = w_t[:]

            def wslice(col, ncols, p0=0, np_=128):
                return bap(wap, col, [[wap.ap[0][0], np_], [1, ncols]]) if p0 == 0 else \
                    bass.AP(tensor=wap.tensor,
                            offset=wap.offset + col + p0 * wap.ap[0][0],
                            ap=[[wap.ap[0][0], np_], [1, ncols]])

            tv_ap = wslice(_WC_TV, 1)                 # [128,1]
            d36_ap = wslice(_WC_D36, APM)             # [128,32]

            with tc.For_i(0, repeats) as _rep:
                nc.sync.dma_start(out=h_t[:], in_=h0_d[:])

                # ================= PHASE A: graph build =================
                posP = sc.tile([128, NBLK, 3], F32, tag="posP")
                nc.sync.dma_start(
                    out=posP[:],
                    in_=bap(pos_d[:], 0, [[3, 128], [384, NBLK], [1, 3]]))
                posB = sc.tile([128, NBLK, APM, 3], F32, tag="posB")
                for b in range(NBLK):
                    nc.sync.dma_start(
                        out=posB[:, b],
                        in_=bap(pos_d[:], 384 * b,
                                [[96, 4], [0, APM], [3, APM], [1, 3]]))
                dif = sc.tile([128, NBLK, APM, 3], F32, tag="dif")
                pP = posP[:]
                nc.vector.tensor_tensor(
                    out=dif[:],
                    in0=bap(pP, 0, [pP.ap[0], [3, NBLK], [0, APM], [1, 3]]),
                    in1=posB[:], op=OP.subtract)
                nc.vector.tensor_tensor(out=dif[:], in0=dif[:], in1=dif[:], op=OP.mult)
                d2m = sc.tile([128, NBLK, APM], F32, tag="d2m")
                nc.vector.tensor_reduce(out=d2m[:], in_=dif[:], axis=AX.X, op=OP.add)

                # clamp to 36 beyond cutoff, force diagonal to 36
                nc.vector.tensor_scalar(out=d2m[:], in0=d2m[:], scalar1=36.0,
                                        scalar2=None, op0=OP.min)
                dd = d2m[:]
                nc.vector.tensor_tensor(
                    out=dd,
                    in0=dd,
                    in1=bap(d36_ap, 0, [d36_ap.ap[0], [0, NBLK], [1, APM]]),
                    op=OP.max)

                # rank among 32 slots; sel = rank < K
                lt = sc.tile([128, NBLK, APM, APM], F32, tag="lt")
                nc.vector.tensor_tensor(
                    out=lt[:],
                    in0=bap(dd, 0, [dd.ap[0], [APM, NBLK], [0, APM], [1, APM]]),
                    in1=bap(dd, 0, [dd.ap[0], [APM, NBLK], [1, APM], [0, APM]]),
                    op=OP.is_lt)
                rk = sc.tile([128, NBLK, APM], F32, tag="rk")
                nc.vector.tensor_reduce(out=rk[:], in_=lt[:], axis=AX.X, op=OP.add)
                sel = rk
                nc.vector.tensor_scalar(out=sel[:], in0=rk[:],
                                        scalar1=float(K) - 0.5, scalar2=None,
                                        op0=OP.is_lt)

                s_t = sc.tile([128, NBLK, APM], F32, tag="s")
                nc.scalar.activation(s_t[:], d2m[:], A.Sqrt)
                dtil = s_t
                nc.vector.scalar_tensor_tensor(
                    out=dtil[:], in0=s_t[:], scalar=-6.0, in1=sel[:],
                    op0=OP.add, op1=OP.mult)
                nc.vector.tensor_scalar(out=dtil[:], in0=dtil[:], scalar1=6.0,
                                        scalar2=None, op0=OP.add)

                # edge e = 32*atom + j; atom = 128*b + p
                nc.sync.dma_start(
                    out=bap(dtil_dram[:], 0, [[APM, 128], [4096, NBLK], [1, APM]]),
                    in_=dtil[:])

                # broadcast back: rows 0:64 hold edges [0,EH), rows 64:128 hold
                # [EH,E); tent coefficients c = relu(1 - |d/TH - t|)
                # c = relu(1 - |d - t*TH|/TH) = max(0, min(1 + q/TH, 1 - q/TH))
                DBC = 4096
                dbc = sc.tile([128, DBC], F32, tag="dbc")
                neg = sc.tile([128, DBC], F32, tag="neg")
                for kk in range(EH // DBC):
                    ks = slice(DBC * kk, DBC * (kk + 1))
                    nc.sync.dma_start(
                        out=dbc[:],
                        in_=bap(dtil_dram[:], DBC * kk, [[EH, 2], [0, T], [1, DBC]]))
                    nc.vector.tensor_scalar(out=dbc[:], in0=dbc[:],
                                            scalar1=tv_ap, scalar2=None,
                                            op0=OP.subtract)
                    nc.vector.tensor_scalar(out=neg[:], in0=dbc[:],
                                            scalar1=-1.0 / TH, scalar2=1.0,
                                            op0=OP.mult, op1=OP.add)
                    nc.vector.tensor_scalar(out=dbc[:], in0=dbc[:],
                                            scalar1=1.0 / TH, scalar2=1.0,
                                            op0=OP.mult, op1=OP.add)
                    nc.vector.tensor_tensor(out=dbc[:], in0=dbc[:], in1=neg[:],
                                            op=OP.min)
                    nc.vector.tensor_scalar(out=neg[:], in0=dbc[:], scalar1=0.0,
                                            scalar2=None, op0=OP.is_gt)
                    nc.vector.tensor_tensor(out=c_t[:, ks], in0=dbc[:], in1=neg[:],
                                            op=OP.mult)

                # ================= PHASE B: interaction layers =================
                for l in range(L):
                    lf0 = l * FEAT
                    ps_x1 = psX.tile([FEAT, NA], F32, tag="psx")
                    for hh in range(2):
                        qs = slice(512 * hh, 512 * (hh + 1))
                        nc.tensor.matmul(ps_x1[:, qs],
                                         wslice(_WC_L1W + lf0, FEAT, 0, FEAT),
                                         h_t[:, qs], start=True, stop=True)
                    nc.vector.tensor_copy(x1s_t[:], ps_x1[:])

                    for cc in range(NCH):
                        half = cc // (NCH // 2)
                        colbase = (cc % (NCH // 2)) * CH
                        ps_e = psE.tile([FEAT, CH], F32, tag="pse")
                        for q2 in range(CH // 512):
                            cs = slice(colbase + 512 * q2, colbase + 512 * (q2 + 1))
                            nc.tensor.matmul(
                                ps_e[:, 512 * q2:512 * (q2 + 1)],
                                wslice(_WC_TBL + lf0, FEAT, 64 * half, T),
                                c_t[64 * half:64 * half + T, cs],
                                start=True, stop=True)
                        a0 = APC * cc
                        msg = sc.tile([FEAT, 2, APM, APM], F32, tag="msg")
                        x1b = x1s_t[:]
                        nc.vector.tensor_tensor(
                            out=msg[:],
                            in0=ps_e[:].rearrange("p (m i j) -> p m i j", m=2, i=APM),
                            in1=bap(x1b, a0, [x1b.ap[0], [APM, 2], [0, APM], [1, APM]]),
                            op=OP.mult)
                        nc.vector.tensor_reduce(
                            out=agg_t[:, a0:a0 + APC],
                            in_=msg[:].rearrange("p m i j -> p (m i) j"),
                            axis=AX.X, op=OP.add)

                    ps_v = psN.tile([FEAT, NA], F32, tag="psn")
                    for hh in range(2):
                        qs = slice(512 * hh, 512 * (hh + 1))
                        nc.tensor.matmul(ps_v[:, qs],
                                         wslice(_WC_L2W + lf0, FEAT, 0, FEAT),
                                         agg_t[:, qs], start=True, stop=True)
                    spe = sc.tile([FEAT, NA], F32, tag="spe")
                    nc.scalar.activation(spe[:], ps_v[:], A.Exp,
                                         bias=wslice(_WC_L2B + l, 1, 0, FEAT))
                    spl = sc.tile([FEAT, NA], F32, tag="spl")
                    nc.scalar.activation(spl[:], spe[:], A.Ln,
                                         bias=half_t[:FEAT], scale=0.5)
                    ps_w = psN.tile([FEAT, NA], F32, tag="psn")
                    for hh in range(2):
                        qs = slice(512 * hh, 512 * (hh + 1))
                        nc.tensor.matmul(ps_w[:, qs],
                                         wslice(_WC_LW + lf0, FEAT, 0, FEAT),
                                         spl[:, qs], start=True, stop=True)
                    nc.vector.scalar_tensor_tensor(
                        out=h_t[:], in0=ps_w[:],
                        scalar=wslice(_WC_LBP + l, 1, 0, FEAT),
                        in1=h_t[:], op0=OP.add, op1=OP.add)

                # ================= PHASE C: readout =================
                ps_r = psN.tile([FEAT, NA], F32, tag="psn")
                for hh in range(2):
                    qs = slice(512 * hh, 512 * (hh + 1))
                    nc.tensor.matmul(ps_r[:H, qs], wslice(_WC_OW1, H, 0, FEAT),
                                     h_t[:, qs], start=True, stop=True)
                re_ = sc.tile([H, NA], F32, tag="re")
                nc.scalar.activation(re_[:], ps_r[:H, :], A.Exp,
                                     bias=wslice(_WC_OB1, 1, 0, H))
                rl = sc.tile([H, NA], F32, tag="rl")
                nc.scalar.activation(rl[:], re_[:], A.Ln,
                                     bias=half_t[:H], scale=0.5)
                ps_o = psE.tile([FEAT, CH], F32, tag="pse")
                for hh in range(2):
                    qs = slice(512 * hh, 512 * (hh + 1))
                    nc.tensor.matmul(ps_o[:1, qs], wslice(_WC_OW2, 1, 0, H),
                                     rl[:, qs], start=True, stop=True)
                pa = sc.tile([1, NA], F32, tag="pa")
                nc.vector.tensor_scalar(out=pa[:], in0=ps_o[:1, :NA],
                                        scalar1=wslice(_WC_OB2, 1, 0, 1),
                                        scalar2=None, op0=OP.add)
                en = sc.tile([1, NM], F32, tag="en")
                nc.vector.tensor_reduce(
                    out=en[:], in_=pa[:].rearrange("p (m i) -> p m i", i=APM),
                    axis=AX.X, op=OP.add)
                nc.sync.dma_start(out=out_d[:].unsqueeze(0), in_=en[:])

    nc.compile()
    return nc


def _filter_exact(l, dv, mlp_w1, mlp_b1, mlp_w2, mlp_b2):
    offset = np.linspace(0.0, CUTOFF, NG)
    coeff = -0.5 / (offset[1] - offset[0]) ** 2
    ea = np.exp(coeff * (dv[:, None] - offset[None, :]) ** 2)
    pre = ea @ mlp_w1[l] + mlp_b1[l]
    W = (np.logaddexp(0, pre) - LOG2) @ mlp_w2[l] + mlp_b2[l]
    cc = 0.5 * (np.cos(dv * np.pi / CUTOFF) + 1.0)
    return W * cc[:, None]


def _prep_inputs(z, pos, ptr, emb, mlp_w1, mlp_b1, mlp_w2, mlp_b2,
                 lin1_w, lin2_w, lin2_b, lin_w, lin_b,
                 out_w1, out_b1, out_w2, out_b2):
    z = np.asarray(z)
    pos = np.ascontiguousarray(np.asarray(pos, dtype=np.float32))
    ptr = np.asarray(ptr)
    assert pos.shape == (N, 3)
    expect = np.arange(0, N + APM, APM)
    assert np.array_equal(ptr.astype(np.int64), expect), "non-uniform molecules unsupported"

    emb = np.asarray(emb, dtype=np.float32)
    mlp_w1 = np.asarray(mlp_w1, dtype=np.float64)
    mlp_b1 = np.asarray(mlp_b1, dtype=np.float64)
    mlp_w2 = np.asarray(mlp_w2, dtype=np.float64)
    mlp_b2 = np.asarray(mlp_b2, dtype=np.float64)
    lin_w_f = np.asarray(lin_w, dtype=np.float32)
    out_w2_f = np.asarray(out_w2, dtype=np.float32)

    # least-squares tent-table fit per layer on a fine grid, last node pinned 0
    fine = np.linspace(0.0, CUTOFF, 4096)
    Cb = np.maximum(0.0, 1.0 - np.abs(fine[:, None] / TH - np.arange(T)[None, :]))
    CbL = Cb[:, :T - 1]
    wpack = np.zeros((128, WC), dtype=np.float32)
    for l in range(L):
        F = _filter_exact(l, fine, mlp_w1, mlp_b1, mlp_w2, mlp_b2)
        TBL, *_ = np.linalg.lstsq(CbL, F, rcond=None)
        TBL = np.vstack([TBL, np.zeros((1, FEAT))]).astype(np.float32)
        wpack[0:T, _WC_TBL + l * FEAT:_WC_TBL + (l + 1) * FEAT] = TBL
        wpack[T:2 * T, _WC_TBL + l * FEAT:_WC_TBL + (l + 1) * FEAT] = TBL
    wpack[:FEAT, _WC_L1W:_WC_L1W + L * FEAT] = \
        np.asarray(lin1_w, np.float32).transpose(1, 0, 2).reshape(FEAT, L * FEAT)
    wpack[:FEAT, _WC_L2W:_WC_L2W + L * FEAT] = \
        np.asarray(lin2_w, np.float32).transpose(1, 0, 2).reshape(FEAT, L * FEAT)
    wpack[:FEAT, _WC_LW:_WC_LW + L * FEAT] = \
        lin_w_f.transpose(1, 0, 2).reshape(FEAT, L * FEAT)
    wpack[:FEAT, _WC_L2B:_WC_L2B + L] = np.asarray(lin2_b, np.float32).T
    wpack[:FEAT, _WC_LBP:_WC_LBP + L] = np.asarray(lin_b, np.float32).T
    wpack[:FEAT, _WC_OW1:_WC_OW1 + H] = np.asarray(out_w1, np.float32)
    wpack[:H, _WC_OW2] = out_w2_f.reshape(H)
    wpack[:H, _WC_OB1] = np.asarray(out_b1, np.float32)
    wpack[0, _WC_OB2] = float(np.asarray(out_b2, np.float32).reshape(()))
    for p in range(128):
        wpack[p, _WC_TV] = float(p % T) * TH
        wpack[p, _WC_D36 + (p % APM)] = 36.0

    in_maps = []
    for c in range(NCORES):
        sl = slice(NA * c, NA * (c + 1))
        h0 = emb[np.asarray(z[sl], dtype=np.int64)].T
        in_maps.append({
            "pos": pos[sl].copy(),
            "h0": np.ascontiguousarray(h0, dtype=np.float32),
            "wpack": wpack,
        })
    return in_maps


def kernel(**inputs) -> np.ndarray:
    from concourse.bass_utils import run_bass_kernel_spmd
    global _COMPILED
    if _COMPILED is None:
        _COMPILED = _build(1)
    nc = _COMPILED
    in_maps = _prep_inputs(**inputs)
    res = run_bass_kernel_spmd(nc, in_maps, list(range(NCORES)))
    out = np.concatenate([res.results[c]["energy"] for c in range(NCORES)])
    return out.astype(np.float32)


if __name__ == "__main__":
    _build(1)
    print("built ok")


# revision 18
# speedup vs baseline: 211.3915x; 7.6869x over previous
"""SchNet forward on 8 Trainium2 NeuronCores (Bass/Tile), data-parallel over molecules.

kernel(**inputs) takes FULL inputs (as produced by setup_inputs) and returns
the FULL [256] float32 per-molecule energies. Inside: shards 256 molecules
into 8 groups of 32 (1024 atoms each), runs an SPMD Bass kernel on cores 0-7,
gathers outputs.

Per core: 1024 atoms, all-pairs 32x32 block distances (E=32768 edge slots);
top-28 selection by rank counting; non-selected / masked edges get distance
CUTOFF=6 exactly, where the filter is exactly zero.

The edge filter network (2-layer MLP on 25 gaussian features x cosine cutoff)
depends only on the scalar edge distance, so it is evaluated host-side on a
dense grid and least-squares-fitted to a T=64-node piecewise-linear (tent)
basis per layer. On device each edge only needs its tent coefficient vector
c[t,e] = relu(1 - |d/h - t|) (built once, reused by all 4 layers) and one
matmul TBL_l @ c per 512 edges. The tent node at d=6 is pinned to 0 so
masked edges contribute exactly nothing.

ssp(x) = softplus(x) - log(2) is computed with a single Softplus activation;
the -log(2) shift is folded into the following linear layer's bias host-side.

The whole per-repetition body sits inside one hardware For_i loop, so
repeated executions reuse the same static instruction stream.
"""

import math
import numpy as np

N = 8192
APM = 32
FEAT = 100
NG = 25
K = 28
L = 4
CUTOFF = 6.0
NCORES = 8
NA = N // NCORES          # atoms per core = 1024
NM = NA // APM            # molecules per core = 32
E = NA * APM              # edge slots per core = 32768
EH = E // 2               # edges per half (on partition rows 0-63 / 64-127)
T = 64                    # tent-basis nodes over [0, 6]
TH = CUTOFF / (T - 1)
NBLK = NA // 128          # 8 atom blocks per core
CH = 2048                 # edges per chunk (PSUM tile)
NCH = E // CH             # 16 chunks
APC = CH // APM           # atoms per chunk = 64
H = FEAT // 2
LOG2 = float(np.log(2.0))

# wpack column layout
_WC_TBL = 0                      # [128, L*FEAT] tent tables (rows 0:64 and 64:128 identical)
_WC_L1W = _WC_TBL + L * FEAT     # [100, L*FEAT]
_WC_L2W = _WC_L1W + L * FEAT
_WC_LW = _WC_L2W + L * FEAT
_WC_L2B = _WC_LW + L * FEAT      # [100, L]
_WC_LBP = _WC_L2B + L            # [100, L]  lin_b - log2*colsum(lin_w)
_WC_OW1 = _WC_LBP + L            # [100, H]
_WC_OW2 = _WC_OW1 + H            # [H, 1]
_WC_OB1 = _WC_OW2 + 1            # [H, 1]
_WC_OB2 = _WC_OB1 + 1            # [1, 1]  out_b2 - log2*sum(out_w2)
_WC_TV = _WC_OB2 + 1             # [128, 1] tent node index (p % 64)
_WC_D36 = _WC_TV + 1             # [128, APM] 36 at j == p%32
WC = _WC_D36 + APM

# build-time options (A/B tested; see work/ablate logs)
OPTS = {
    "bf16mm": True,     # bf16 tent-table matmuls (c_t + table in bf16)
    "poolred": False,   # Pool-engine windowed mean for agg (table pre-scaled x32)
    "nox1copy": False,  # read x1 directly from PSUM in the msg multiply
    "fp16bcast": True,  # round-trip dtil through DRAM as fp16 (halves broadcast)
    "psmsg": False,     # msg multiply in place in PSUM, reduce from PSUM
    "redgrp": 1,        # chunks per agg reduce (1, 2, or 4)
    "tsmax": True,      # single tensor_scalar max for tent relu
}

_COMPILED = None


def _build(repeats: int = 1):
    import concourse.bass as bass
    import concourse.mybir as mybir
    import concourse.tile as tile
    from concourse import bacc

    dt = mybir.dt
    F32 = dt.float32
    F32R = dt.float32r
    A = mybir.ActivationFunctionType
    OP = mybir.AluOpType
    AX = mybir.AxisListType
    LF = L * FEAT

    nc = bacc.Bacc()

    pos_d = nc.dram_tensor("pos", [NA, 3], F32, kind="ExternalInput")
    h0_d = nc.dram_tensor("h0", [FEAT, NA], F32, kind="ExternalInput")
    wpack_d = nc.dram_tensor("wpack", [128, WC], F32, kind="ExternalInput")
    out_d = nc.dram_tensor("energy", [NM], F32, kind="ExternalOutput")
    dtil_dram = nc.dram_tensor("dtil_lin", [E],
                               dt.float16 if OPTS["fp16bcast"] else F32)

    def bap(a, off, dims):
        return bass.AP(tensor=a.tensor, offset=a.offset + off, ap=dims)

    with tile.TileContext(nc) as tc:
        import contextlib
        ctx = contextlib.ExitStack()
        with ctx:
            wp = ctx.enter_context(tc.tile_pool(name="wp", bufs=1))
            rp = ctx.enter_context(tc.tile_pool(name="rp", bufs=1))
            sc = ctx.enter_context(tc.tile_pool(name="sc", bufs=1))
            psX = ctx.enter_context(tc.tile_pool(name="psX", bufs=1, space="PSUM"))
            psE = ctx.enter_context(tc.tile_pool(name="psE", bufs=1, space="PSUM"))
            psN = ctx.enter_context(tc.tile_pool(name="psN", bufs=1, space="PSUM"))

            w_t = wp.tile([128, WC], F32, tag="wpack")
            nc.sync.dma_start(out=w_t[:], in_=wpack_d[:])
            half_t = wp.tile([128, 1], F32, tag="half")
            nc.vector.memset(half_t[:], 0.5)

            # persistent across reps (recomputed inside each rep)
            h_t = rp.tile([FEAT, NA], F32, tag="h")
            x1s_t = rp.tile([FEAT, NA], F32, tag="x1s")
            agg_t = rp.tile([FEAT, NA], F32, tag="agg")
            c_t = rp.tile([128, EH], dt.bfloat16 if OPTS["bf16mm"] else F32,
                          tag="tentc")

            wap = w_t[:]

            def wslice(col, ncols, p0=0, np_=128):
                return bap(wap, col, [[wap.ap[0][0], np_], [1, ncols]]) if p0 == 0 else \
                    bass.AP(tensor=wap.tensor,
                            offset=wap.offset + col + p0 * wap.ap[0][0],
                            ap=[[wap.ap[0][0], np_], [1, ncols]])

            tv_ap = wslice(_WC_TV, 1)                 # [128,1]
            d36_ap = wslice(_WC_D36, APM)             # [128,32]

            if OPTS["bf16mm"]:
                tblb_t = wp.tile([128, L * FEAT], dt.bfloat16, tag="tblb")
                nc.vector.tensor_copy(tblb_t[:], wslice(_WC_TBL, L * FEAT))

            WARMUP = 64
            with tc.For_i(0, repeats + WARMUP) as _rep:
                nc.sync.dma_start(out=h_t[:], in_=h0_d[:])

                # ================= PHASE A: graph build =================
                posP = sc.tile([128, NBLK, 3], F32, tag="posP")
                nc.sync.dma_start(
                    out=posP[:],
                    in_=bap(pos_d[:], 0, [[3, 128], [384, NBLK], [1, 3]]))
                posB = sc.tile([128, NBLK, APM, 3], F32, tag="posB")
                for b in range(NBLK):
                    nc.sync.dma_start(
                        out=posB[:, b],
                        in_=bap(pos_d[:], 384 * b,
                                [[96, 4], [0, APM], [3, APM], [1, 3]]))
                dif = sc.tile([128, NBLK, APM, 3], F32, tag="dif")
                pP = posP[:]
                nc.vector.tensor_tensor(
                    out=dif[:],
                    in0=bap(pP, 0, [pP.ap[0], [3, NBLK], [0, APM], [1, 3]]),
                    in1=posB[:], op=OP.subtract)
                nc.vector.tensor_tensor(out=dif[:], in0=dif[:], in1=dif[:], op=OP.mult)
                d2m = sc.tile([128, NBLK, APM], F32, tag="d2m")
                nc.vector.tensor_reduce(out=d2m[:], in_=dif[:], axis=AX.X, op=OP.add)

                # clamp to 36 beyond cutoff, force diagonal to 36
                nc.vector.tensor_scalar(out=d2m[:], in0=d2m[:], scalar1=36.0,
                                        scalar2=None, op0=OP.min)
                dd = d2m[:]
                nc.vector.tensor_tensor(
                    out=dd,
                    in0=dd,
                    in1=bap(d36_ap, 0, [d36_ap.ap[0], [0, NBLK], [1, APM]]),
                    op=OP.max)

                # rank among 32 slots; sel = rank < K
                lt = sc.tile([128, NBLK, APM, APM], F32, tag="lt")
                nc.vector.tensor_tensor(
                    out=lt[:],
                    in0=bap(dd, 0, [dd.ap[0], [APM, NBLK], [0, APM], [1, APM]]),
                    in1=bap(dd, 0, [dd.ap[0], [APM, NBLK], [1, APM], [0, APM]]),
                    op=OP.is_lt)
                rk = sc.tile([128, NBLK, APM], F32, tag="rk")
                nc.vector.tensor_reduce(out=rk[:], in_=lt[:], axis=AX.X, op=OP.add)
                sel = rk
                nc.vector.tensor_scalar(out=sel[:], in0=rk[:],
                                        scalar1=float(K) - 0.5, scalar2=None,
                                        op0=OP.is_lt)

                s_t = sc.tile([128, NBLK, APM], F32, tag="s")
                nc.scalar.activation(s_t[:], d2m[:], A.Sqrt)
                dtil = s_t
                nc.vector.scalar_tensor_tensor(
                    out=dtil[:], in0=s_t[:], scalar=-6.0, in1=sel[:],
                    op0=OP.add, op1=OP.mult)
                nc.vector.tensor_scalar(out=dtil[:], in0=dtil[:], scalar1=6.0,
                                        scalar2=None, op0=OP.add)

                # edge e = 32*atom + j; atom = 128*b + p
                if OPTS["fp16bcast"]:
                    dt16 = sc.tile([128, NBLK, APM], dt.float16, tag="dt16")
                    nc.vector.tensor_copy(dt16[:], dtil[:])
                    stile = dt16
                else:
                    stile = dtil
                nc.sync.dma_start(
                    out=bap(dtil_dram[:], 0, [[APM, 128], [4096, NBLK], [1, APM]]),
                    in_=stile[:])

                # broadcast back: rows 0:64 hold edges [0,EH), rows 64:128 hold
                # [EH,E); tent coefficients c = relu(1 - |d/TH - t|)
                # c = relu(1 - |d - t*TH|/TH) = max(0, min(1 + q/TH, 1 - q/TH))
                DBC = 4096
                dbc = sc.tile([128, DBC], F32, tag="dbc")
                neg = sc.tile([128, DBC], F32, tag="neg")
                if OPTS["fp16bcast"]:
                    dbc16 = sc.tile([128, DBC], dt.float16, tag="dbc16")
                for kk in range(EH // DBC):
                    ks = slice(DBC * kk, DBC * (kk + 1))
                    ld = dbc16 if OPTS["fp16bcast"] else dbc
                    nc.sync.dma_start(
                        out=ld[:],
                        in_=bap(dtil_dram[:], DBC * kk, [[EH, 2], [0, T], [1, DBC]]))
                    nc.vector.tensor_scalar(out=dbc[:], in0=ld[:],
                                            scalar1=tv_ap, scalar2=None,
                                            op0=OP.subtract)
                    nc.vector.tensor_scalar(out=neg[:], in0=dbc[:],
                                            scalar1=-1.0 / TH, scalar2=1.0,
                                            op0=OP.mult, op1=OP.add)
                    nc.vector.tensor_scalar(out=dbc[:], in0=dbc[:],
                                            scalar1=1.0 / TH, scalar2=1.0,
                                            op0=OP.mult, op1=OP.add)
                    nc.vector.tensor_tensor(out=dbc[:], in0=dbc[:], in1=neg[:],
                                            op=OP.min)
                    if OPTS["tsmax"]:
                        nc.vector.tensor_scalar(out=c_t[:, ks], in0=dbc[:],
                                                scalar1=0.0, scalar2=None,
                                                op0=OP.max)
                    else:
                        nc.vector.tensor_scalar(out=neg[:], in0=dbc[:], scalar1=0.0,
                                                scalar2=None, op0=OP.is_gt)
                        nc.vector.tensor_tensor(out=c_t[:, ks], in0=dbc[:], in1=neg[:],
                                                op=OP.mult)

                # ================= PHASE B: interaction layers =================
                for l in range(L):
                    lf0 = l * FEAT
                    ps_x1 = psX.tile([FEAT, NA], F32, tag="psx")
                    for hh in range(2):
                        qs = slice(512 * hh, 512 * (hh + 1))
                        nc.tensor.matmul(ps_x1[:, qs],
                                         wslice(_WC_L1W + lf0, FEAT, 0, FEAT),
                                         h_t[:, qs], start=True, stop=True)
                    if not OPTS["nox1copy"]:
                        nc.vector.tensor_copy(x1s_t[:], ps_x1[:])

                    for cc in range(NCH):
                        half = cc // (NCH // 2)
                        colbase = (cc % (NCH // 2)) * CH
                        ps_e = psE.tile([FEAT, CH], F32, tag="pse")
                        for q2 in range(CH // 512):
                            cs = slice(colbase + 512 * q2, colbase + 512 * (q2 + 1))
                            stat = (tblb_t[64 * half:64 * half + T, lf0:lf0 + FEAT]
                                    if OPTS["bf16mm"] else
                                    wslice(_WC_TBL + lf0, FEAT, 64 * half, T))
                            nc.tensor.matmul(
                                ps_e[:, 512 * q2:512 * (q2 + 1)],
                                stat,
                                c_t[64 * half:64 * half + T, cs],
                                start=True, stop=True)
                        a0 = APC * cc
                        x1b = ps_x1[:] if OPTS["nox1copy"] else x1s_t[:]
                        G = OPTS["redgrp"]
                        msg = sc.tile([FEAT, G * CH], F32, tag="msg")
                        gslot = (cc % G) * CH
                        nc.vector.tensor_tensor(
                            out=msg[:, gslot:gslot + CH].rearrange(
                                "p (m i j) -> p m i j", m=2, i=APM),
                            in0=ps_e[:].rearrange("p (m i j) -> p m i j", m=2, i=APM),
                            in1=bap(x1b, a0, [x1b.ap[0], [APM, 2], [0, APM], [1, APM]]),
                            op=OP.mult)
                        if cc % G == G - 1:
                            ga0 = APC * (cc - G + 1)
                            nc.vector.tensor_reduce(
                                out=agg_t[:, ga0:ga0 + G * APC],
                                in_=msg[:].rearrange("p (a j) -> p a j", j=APM),
                                axis=AX.X, op=OP.add)

                    ps_v = psN.tile([FEAT, NA], F32, tag="psn")
                    for hh in range(2):
                        qs = slice(512 * hh, 512 * (hh + 1))
                        nc.tensor.matmul(ps_v[:, qs],
                                         wslice(_WC_L2W + lf0, FEAT, 0, FEAT),
                                         agg_t[:, qs], start=True, stop=True)
                    spe = sc.tile([FEAT, NA], F32, tag="spe")
                    nc.scalar.activation(spe[:], ps_v[:], A.Exp,
                                         bias=wslice(_WC_L2B + l, 1, 0, FEAT))
                    spl = sc.tile([FEAT, NA], F32, tag="spl")
                    nc.scalar.activation(spl[:], spe[:], A.Ln,
                                         bias=half_t[:FEAT], scale=0.5)
                    ps_w = psN.tile([FEAT, NA], F32, tag="psn")
                    for hh in range(2):
                        qs = slice(512 * hh, 512 * (hh + 1))
                        nc.tensor.matmul(ps_w[:, qs],
                                         wslice(_WC_LW + lf0, FEAT, 0, FEAT),
                                         spl[:, qs], start=True, stop=True)
                    nc.vector.scalar_tensor_tensor(
                        out=h_t[:], in0=ps_w[:],
                        scalar=wslice(_WC_LBP + l, 1, 0, FEAT),
                        in1=h_t[:], op0=OP.add, op1=OP.add)

                # ================= PHASE C: readout =================
                ps_r = psN.tile([FEAT, NA], F32, tag="psn")
                for hh in range(2):
                    qs = slice(512 * hh, 512 * (hh + 1))
                    nc.tensor.matmul(ps_r[:H, qs], wslice(_WC_OW1, H, 0, FEAT),
                                     h_t[:, qs], start=True, stop=True)
                re_ = sc.tile([H, NA], F32, tag="re")
                nc.scalar.activation(re_[:], ps_r[:H, :], A.Exp,
                                     bias=wslice(_WC_OB1, 1, 0, H))
                rl = sc.tile([H, NA], F32, tag="rl")
                nc.scalar.activation(rl[:], re_[:], A.Ln,
                                     bias=half_t[:H], scale=0.5)
                ps_o = psE.tile([FEAT, CH], F32, tag="pse")
                for hh in range(2):
                    qs = slice(512 * hh, 512 * (hh + 1))
                    nc.tensor.matmul(ps_o[:1, qs], wslice(_WC_OW2, 1, 0, H),
                                     rl[:, qs], start=True, stop=True)
                pa = sc.tile([1, NA], F32, tag="pa")
                nc.vector.tensor_scalar(out=pa[:], in0=ps_o[:1, :NA],
                                        scalar1=wslice(_WC_OB2, 1, 0, 1),
                                        scalar2=None, op0=OP.add)
                en = sc.tile([1, NM], F32, tag="en")
                nc.vector.tensor_reduce(
                    out=en[:], in_=pa[:].rearrange("p (m i) -> p m i", i=APM),
                    axis=AX.X, op=OP.add)
                nc.sync.dma_start(out=out_d[:].unsqueeze(0), in_=en[:])

    nc.compile()
    return nc


def _filter_exact(l, dv, mlp_w1, mlp_b1, mlp_w2, mlp_b2):
    offset = np.linspace(0.0, CUTOFF, NG)
    coeff = -0.5 / (offset[1] - offset[0]) ** 2
    ea = np.exp(coeff * (dv[:, None] - offset[None, :]) ** 2)
    pre = ea @ mlp_w1[l] + mlp_b1[l]
    W = (np.logaddexp(0, pre) - LOG2) @ mlp_w2[l] + mlp_b2[l]
    cc = 0.5 * (np.cos(dv * np.pi / CUTOFF) + 1.0)
    return W * cc[:, None]


def _prep_inputs(z, pos, ptr, emb, mlp_w1, mlp_b1, mlp_w2, mlp_b2,
                 lin1_w, lin2_w, lin2_b, lin_w, lin_b,
                 out_w1, out_b1, out_w2, out_b2):
    z = np.asarray(z)
    pos = np.ascontiguousarray(np.asarray(pos, dtype=np.float32))
    ptr = np.asarray(ptr)
    assert pos.shape == (N, 3)
    expect = np.arange(0, N + APM, APM)
    assert np.array_equal(ptr.astype(np.int64), expect), "non-uniform molecules unsupported"

    emb = np.asarray(emb, dtype=np.float32)
    mlp_w1 = np.asarray(mlp_w1, dtype=np.float64)
    mlp_b1 = np.asarray(mlp_b1, dtype=np.float64)
    mlp_w2 = np.asarray(mlp_w2, dtype=np.float64)
    mlp_b2 = np.asarray(mlp_b2, dtype=np.float64)
    lin_w_f = np.asarray(lin_w, dtype=np.float32)
    out_w2_f = np.asarray(out_w2, dtype=np.float32)

    # least-squares tent-table fit per layer on a fine grid, last node pinned 0
    fine = np.linspace(0.0, CUTOFF, 4096)
    Cb = np.maximum(0.0, 1.0 - np.abs(fine[:, None] / TH - np.arange(T)[None, :]))
    CbL = Cb[:, :T - 1]
    wpack = np.zeros((128, WC), dtype=np.float32)
    for l in range(L):
        F = _filter_exact(l, fine, mlp_w1, mlp_b1, mlp_w2, mlp_b2)
        TBL, *_ = np.linalg.lstsq(CbL, F, rcond=None)
        TBL = np.vstack([TBL, np.zeros((1, FEAT))]).astype(np.float32)
        if OPTS["poolred"]:
            TBL = TBL * float(APM)
        wpack[0:T, _WC_TBL + l * FEAT:_WC_TBL + (l + 1) * FEAT] = TBL
        wpack[T:2 * T, _WC_TBL + l * FEAT:_WC_TBL + (l + 1) * FEAT] = TBL
    wpack[:FEAT, _WC_L1W:_WC_L1W + L * FEAT] = \
        np.asarray(lin1_w, np.float32).transpose(1, 0, 2).reshape(FEAT, L * FEAT)
    wpack[:FEAT, _WC_L2W:_WC_L2W + L * FEAT] = \
        np.asarray(lin2_w, np.float32).transpose(1, 0, 2).reshape(FEAT, L * FEAT)
    wpack[:FEAT, _WC_LW:_WC_LW + L * FEAT] = \
        lin_w_f.transpose(1, 0, 2).reshape(FEAT, L * FEAT)
    wpack[:FEAT, _WC_L2B:_WC_L2B + L] = np.asarray(lin2_b, np.float32).T
    wpack[:FEAT, _WC_LBP:_WC_LBP + L] = np.asarray(lin_b, np.float32).T
    wpack[:FEAT, _WC_OW1:_WC_OW1 + H] = np.asarray(out_w1, np.float32)
    wpack[:H, _WC_OW2] = out_w2_f.reshape(H)
    wpack[:H, _WC_OB1] = np.asarray(out_b1, np.float32)
    wpack[0, _WC_OB2] = float(np.asarray(out_b2, np.float32).reshape(()))
    for p in range(128):
        wpack[p, _WC_TV] = float(p % T) * TH
        wpack[p, _WC_D36 + (p % APM)] = 36.0

    in_maps = []
    for c in range(NCORES):
        sl = slice(NA * c, NA * (c + 1))
        h0 = emb[np.asarray(z[sl], dtype=np.int64)].T
        in_maps.append({
            "pos": pos[sl].copy(),
            "h0": np.ascontiguousarray(h0, dtype=np.float32),
            "wpack": wpack,
        })
    return in_maps


def kernel(**inputs) -> np.ndarray:
    from concourse.bass_utils import run_bass_kernel_spmd
    global _COMPILED
    if _COMPILED is None:
        _COMPILED = _build(1)
    nc = _COMPILED
    in_maps = _prep_inputs(**inputs)
    res = run_bass_kernel_spmd(nc, in_maps, list(range(NCORES)))
    out = np.concatenate([res.results[c]["energy"] for c in range(NCORES)])
    return out.astype(np.float32)


if __name__ == "__main__":
    _build(1)
    print("built ok")


# revision 19
# speedup vs baseline: 215.2603x; 1.0183x over previous
"""SchNet forward on 8 Trainium2 NeuronCores (Bass/Tile), data-parallel over molecules.

kernel(**inputs) takes FULL inputs (as produced by setup_inputs) and returns
the FULL [256] float32 per-molecule energies. Inside: shards 256 molecules
into 8 groups of 32 (1024 atoms each), runs an SPMD Bass kernel on cores 0-7,
gathers outputs.

Per core: 1024 atoms, all-pairs 32x32 block distances (E=32768 edge slots);
top-28 selection by rank counting; non-selected / masked edges get distance
CUTOFF=6 exactly, where the filter is exactly zero.

The edge filter network (2-layer MLP on 25 gaussian features x cosine cutoff)
depends only on the scalar edge distance, so it is evaluated host-side on a
dense grid and least-squares-fitted to a T=64-node piecewise-linear (tent)
basis per layer. On device each edge only needs its tent coefficient vector
c[t,e] = relu(1 - |d/h - t|) (built once, reused by all 4 layers) and one
matmul TBL_l @ c per 512 edges. The tent node at d=6 is pinned to 0 so
masked edges contribute exactly nothing.

ssp(x) = softplus(x) - log(2) is computed with a single Softplus activation;
the -log(2) shift is folded into the following linear layer's bias host-side.

The whole per-repetition body sits inside one hardware For_i loop, so
repeated executions reuse the same static instruction stream.
"""

import math
import numpy as np

N = 8192
APM = 32
FEAT = 100
NG = 25
K = 28
L = 4
CUTOFF = 6.0
NCORES = 8
NA = N // NCORES          # atoms per core = 1024
NM = NA // APM            # molecules per core = 32
E = NA * APM              # edge slots per core = 32768
EH = E // 2               # edges per half (on partition rows 0-63 / 64-127)
T = 64                    # tent-basis nodes over [0, 6]
TH = CUTOFF / (T - 1)
NBLK = NA // 128          # 8 atom blocks per core
CH = 2048                 # edges per chunk (PSUM tile)
NCH = E // CH             # 16 chunks
APC = CH // APM           # atoms per chunk = 64
H = FEAT // 2
LOG2 = float(np.log(2.0))

# wpack column layout
_WC_TBL = 0                      # [128, L*FEAT] tent tables (rows 0:64 and 64:128 identical)
_WC_L1W = _WC_TBL + L * FEAT     # [100, L*FEAT]
_WC_L2W = _WC_L1W + L * FEAT
_WC_LW = _WC_L2W + L * FEAT
_WC_L2B = _WC_LW + L * FEAT      # [100, L]
_WC_LBP = _WC_L2B + L            # [100, L]  lin_b - log2*colsum(lin_w)
_WC_OW1 = _WC_LBP + L            # [100, H]
_WC_OW2 = _WC_OW1 + H            # [H, 1]
_WC_OB1 = _WC_OW2 + 1            # [H, 1]
_WC_OB2 = _WC_OB1 + 1            # [1, 1]  out_b2 - log2*sum(out_w2)
_WC_TV = _WC_OB2 + 1             # [128, 1] tent node index (p % 64)
_WC_D36 = _WC_TV + 1             # [128, APM] 36 at j == p%32
WC = _WC_D36 + APM

# build-time options (A/B tested; see work/ablate logs)
OPTS = {
    "bf16mm": True,     # bf16 tent-table matmuls (c_t + table in bf16)
    "poolred": False,   # Pool-engine windowed mean for agg (table pre-scaled x32)
    "nox1copy": False,  # read x1 directly from PSUM in the msg multiply
    "fp16bcast": True,  # round-trip dtil through DRAM as fp16 (halves broadcast)
    "psmsg": False,     # msg multiply in place in PSUM, reduce from PSUM
    "redgrp": 1,        # chunks per agg reduce (1, 2, or 4)
    "tsmax": True,      # single tensor_scalar max for tent relu
}

_COMPILED = None


def _build(repeats: int = 1):
    import concourse.bass as bass
    import concourse.mybir as mybir
    import concourse.tile as tile
    from concourse import bacc

    dt = mybir.dt
    F32 = dt.float32
    F32R = dt.float32r
    A = mybir.ActivationFunctionType
    OP = mybir.AluOpType
    AX = mybir.AxisListType
    LF = L * FEAT

    nc = bacc.Bacc()

    pos_d = nc.dram_tensor("pos", [NA, 3], F32, kind="ExternalInput")
    h0_d = nc.dram_tensor("h0", [FEAT, NA], F32, kind="ExternalInput")
    wpack_d = nc.dram_tensor("wpack", [128, WC], F32, kind="ExternalInput")
    out_d = nc.dram_tensor("energy", [NM], F32, kind="ExternalOutput")
    dtil_dram = nc.dram_tensor("dtil_lin", [E],
                               dt.float16 if OPTS["fp16bcast"] else F32)

    def bap(a, off, dims):
        return bass.AP(tensor=a.tensor, offset=a.offset + off, ap=dims)

    with tile.TileContext(nc) as tc:
        import contextlib
        ctx = contextlib.ExitStack()
        with ctx:
            wp = ctx.enter_context(tc.tile_pool(name="wp", bufs=1))
            rp = ctx.enter_context(tc.tile_pool(name="rp", bufs=1))
            sc = ctx.enter_context(tc.tile_pool(name="sc", bufs=1))
            psX = ctx.enter_context(tc.tile_pool(name="psX", bufs=1, space="PSUM"))
            psE = ctx.enter_context(tc.tile_pool(name="psE", bufs=1, space="PSUM"))
            psN = ctx.enter_context(tc.tile_pool(name="psN", bufs=1, space="PSUM"))

            w_t = wp.tile([128, WC], F32, tag="wpack")
            nc.sync.dma_start(out=w_t[:], in_=wpack_d[:])
            half_t = wp.tile([128, 1], F32, tag="half")
            nc.vector.memset(half_t[:], 0.5)

            # persistent across reps (recomputed inside each rep)
            h_t = rp.tile([FEAT, NA], F32, tag="h")
            x1s_t = rp.tile([FEAT, NA], F32, tag="x1s")
            agg_t = rp.tile([FEAT, NA], F32, tag="agg")
            c_t = rp.tile([128, EH], dt.bfloat16 if OPTS["bf16mm"] else F32,
                          tag="tentc")

            wap = w_t[:]

            def wslice(col, ncols, p0=0, np_=128):
                return bap(wap, col, [[wap.ap[0][0], np_], [1, ncols]]) if p0 == 0 else \
                    bass.AP(tensor=wap.tensor,
                            offset=wap.offset + col + p0 * wap.ap[0][0],
                            ap=[[wap.ap[0][0], np_], [1, ncols]])

            tv_ap = wslice(_WC_TV, 1)                 # [128,1]
            d36_ap = wslice(_WC_D36, APM)             # [128,32]

            if OPTS["bf16mm"]:
                tblb_t = wp.tile([128, L * FEAT], dt.bfloat16, tag="tblb")
                nc.vector.tensor_copy(tblb_t[:], wslice(_WC_TBL, L * FEAT))

            WARMUP = 64
            with tc.For_i(0, repeats + WARMUP) as _rep:
                nc.sync.dma_start(out=h_t[:], in_=h0_d[:])

                # ================= PHASE A: graph build =================
                posP = sc.tile([128, NBLK, 3], F32, tag="posP")
                nc.sync.dma_start(
                    out=posP[:],
                    in_=bap(pos_d[:], 0, [[3, 128], [384, NBLK], [1, 3]]))
                posB = sc.tile([128, NBLK, APM, 3], F32, tag="posB")
                for b in range(NBLK):
                    nc.sync.dma_start(
                        out=posB[:, b],
                        in_=bap(pos_d[:], 384 * b,
                                [[96, 4], [0, APM], [3, APM], [1, 3]]))
                dif = sc.tile([128, NBLK, APM, 3], F32, tag="dif")
                pP = posP[:]
                nc.vector.tensor_tensor(
                    out=dif[:],
                    in0=bap(pP, 0, [pP.ap[0], [3, NBLK], [0, APM], [1, 3]]),
                    in1=posB[:], op=OP.subtract)
                nc.vector.tensor_tensor(out=dif[:], in0=dif[:], in1=dif[:], op=OP.mult)
                d2m = sc.tile([128, NBLK, APM], F32, tag="d2m")
                nc.vector.tensor_reduce(out=d2m[:], in_=dif[:], axis=AX.X, op=OP.add)

                # clamp to 36 beyond cutoff, force diagonal to 36
                nc.vector.tensor_scalar(out=d2m[:], in0=d2m[:], scalar1=36.0,
                                        scalar2=None, op0=OP.min)
                dd = d2m[:]
                nc.vector.tensor_tensor(
                    out=dd,
                    in0=dd,
                    in1=bap(d36_ap, 0, [d36_ap.ap[0], [0, NBLK], [1, APM]]),
                    op=OP.max)

                # rank among 32 slots; sel = rank < K
                lt = sc.tile([128, NBLK, APM, APM], dt.bfloat16, tag="lt")
                nc.vector.tensor_tensor(
                    out=lt[:],
                    in0=bap(dd, 0, [dd.ap[0], [APM, NBLK], [0, APM], [1, APM]]),
                    in1=bap(dd, 0, [dd.ap[0], [APM, NBLK], [1, APM], [0, APM]]),
                    op=OP.is_lt)
                rk = sc.tile([128, NBLK, APM], F32, tag="rk")
                nc.vector.tensor_reduce(out=rk[:], in_=lt[:], axis=AX.X, op=OP.add)
                sel = rk
                nc.vector.tensor_scalar(out=sel[:], in0=rk[:],
                                        scalar1=float(K) - 0.5, scalar2=None,
                                        op0=OP.is_lt)

                s_t = sc.tile([128, NBLK, APM], F32, tag="s")
                nc.scalar.activation(s_t[:], d2m[:], A.Sqrt)
                dtil = s_t
                nc.vector.scalar_tensor_tensor(
                    out=dtil[:], in0=s_t[:], scalar=-6.0, in1=sel[:],
                    op0=OP.add, op1=OP.mult)
                nc.vector.tensor_scalar(out=dtil[:], in0=dtil[:],
                                        scalar1=1.0 / TH, scalar2=6.0 / TH,
                                        op0=OP.mult, op1=OP.add)

                # edge e = 32*atom + j; atom = 128*b + p
                if OPTS["fp16bcast"]:
                    dt16 = sc.tile([128, NBLK, APM], dt.float16, tag="dt16")
                    nc.vector.tensor_copy(dt16[:], dtil[:])
                    stile = dt16
                else:
                    stile = dtil
                nc.sync.dma_start(
                    out=bap(dtil_dram[:], 0, [[APM, 128], [4096, NBLK], [1, APM]]),
                    in_=stile[:])

                # broadcast back: rows 0:64 hold edges [0,EH), rows 64:128 hold
                # [EH,E); tent coefficients c = relu(1 - |d/TH - t|)
                # r = d/TH broadcast; u2 = r - (t-1) = 1 + (r-t);
                # c = relu(min(2 - u2, u2)) = relu(1 - |r - t|)
                DBC = 8192
                dbc = sc.tile([128, DBC], F32, tag="dbc")
                neg = sc.tile([128, DBC], F32, tag="neg")
                if OPTS["fp16bcast"]:
                    dbc16 = sc.tile([128, DBC], dt.float16, tag="dbc16")
                for kk in range(EH // DBC):
                    ks = slice(DBC * kk, DBC * (kk + 1))
                    ld = dbc16 if OPTS["fp16bcast"] else dbc
                    nc.sync.dma_start(
                        out=ld[:],
                        in_=bap(dtil_dram[:], DBC * kk, [[EH, 2], [0, T], [1, DBC]]))
                    nc.vector.tensor_scalar(out=dbc[:], in0=ld[:],
                                            scalar1=tv_ap, scalar2=None,
                                            op0=OP.subtract)
                    nc.vector.tensor_scalar(out=neg[:], in0=dbc[:],
                                            scalar1=-1.0, scalar2=2.0,
                                            op0=OP.mult, op1=OP.add)
                    nc.vector.tensor_tensor(out=dbc[:], in0=dbc[:], in1=neg[:],
                                            op=OP.min)
                    nc.vector.tensor_scalar(out=c_t[:, ks], in0=dbc[:],
                                            scalar1=0.0, scalar2=None,
                                            op0=OP.max)

                # ================= PHASE B: interaction layers =================
                for l in range(L):
                    lf0 = l * FEAT
                    ps_x1 = psX.tile([FEAT, NA], F32, tag="psx")
                    for hh in range(2):
                        qs = slice(512 * hh, 512 * (hh + 1))
                        nc.tensor.matmul(ps_x1[:, qs],
                                         wslice(_WC_L1W + lf0, FEAT, 0, FEAT),
                                         h_t[:, qs], start=True, stop=True)
                    if not OPTS["nox1copy"]:
                        nc.vector.tensor_copy(x1s_t[:], ps_x1[:])

                    for cc in range(NCH):
                        half = cc // (NCH // 2)
                        colbase = (cc % (NCH // 2)) * CH
                        ps_e = psE.tile([FEAT, CH], F32, tag="pse")
                        for q2 in range(CH // 512):
                            cs = slice(colbase + 512 * q2, colbase + 512 * (q2 + 1))
                            stat = (tblb_t[64 * half:64 * half + T, lf0:lf0 + FEAT]
                                    if OPTS["bf16mm"] else
                                    wslice(_WC_TBL + lf0, FEAT, 64 * half, T))
                            nc.tensor.matmul(
                                ps_e[:, 512 * q2:512 * (q2 + 1)],
                                stat,
                                c_t[64 * half:64 * half + T, cs],
                                start=True, stop=True)
                        a0 = APC * cc
                        x1b = ps_x1[:] if OPTS["nox1copy"] else x1s_t[:]
                        G = OPTS["redgrp"]
                        msg = sc.tile([FEAT, G * CH], F32, tag="msg")
                        gslot = (cc % G) * CH
                        nc.vector.tensor_tensor(
                            out=msg[:, gslot:gslot + CH].rearrange(
                                "p (m i j) -> p m i j", m=2, i=APM),
                            in0=ps_e[:].rearrange("p (m i j) -> p m i j", m=2, i=APM),
                            in1=bap(x1b, a0, [x1b.ap[0], [APM, 2], [0, APM], [1, APM]]),
                            op=OP.mult)
                        if cc % G == G - 1:
                            ga0 = APC * (cc - G + 1)
                            nc.vector.tensor_reduce(
                                out=agg_t[:, ga0:ga0 + G * APC],
                                in_=msg[:].rearrange("p (a j) -> p a j", j=APM),
                                axis=AX.X, op=OP.add)

                    ps_v = psN.tile([FEAT, NA], F32, tag="psn")
                    for hh in range(2):
                        qs = slice(512 * hh, 512 * (hh + 1))
                        nc.tensor.matmul(ps_v[:, qs],
                                         wslice(_WC_L2W + lf0, FEAT, 0, FEAT),
                                         agg_t[:, qs], start=True, stop=True)
                    spe = sc.tile([FEAT, NA], F32, tag="spe")
                    nc.scalar.activation(spe[:], ps_v[:], A.Exp,
                                         bias=wslice(_WC_L2B + l, 1, 0, FEAT))
                    spl = sc.tile([FEAT, NA], F32, tag="spl")
                    nc.scalar.activation(spl[:], spe[:], A.Ln,
                                         bias=half_t[:FEAT], scale=0.5)
                    ps_w = psN.tile([FEAT, NA], F32, tag="psn")
                    for hh in range(2):
                        qs = slice(512 * hh, 512 * (hh + 1))
                        nc.tensor.matmul(ps_w[:, qs],
                                         wslice(_WC_LW + lf0, FEAT, 0, FEAT),
                                         spl[:, qs], start=True, stop=True)
                    nc.vector.scalar_tensor_tensor(
                        out=h_t[:], in0=ps_w[:],
                        scalar=wslice(_WC_LBP + l, 1, 0, FEAT),
                        in1=h_t[:], op0=OP.add, op1=OP.add)

                # ================= PHASE C: readout =================
                ps_r = psN.tile([FEAT, NA], F32, tag="psn")
                for hh in range(2):
                    qs = slice(512 * hh, 512 * (hh + 1))
                    nc.tensor.matmul(ps_r[:H, qs], wslice(_WC_OW1, H, 0, FEAT),
                                     h_t[:, qs], start=True, stop=True)
                re_ = sc.tile([H, NA], F32, tag="re")
                nc.scalar.activation(re_[:], ps_r[:H, :], A.Exp,
                                     bias=wslice(_WC_OB1, 1, 0, H))
                rl = sc.tile([H, NA], F32, tag="rl")
                nc.scalar.activation(rl[:], re_[:], A.Ln,
                                     bias=half_t[:H], scale=0.5)
                ps_o = psE.tile([FEAT, CH], F32, tag="pse")
                for hh in range(2):
                    qs = slice(512 * hh, 512 * (hh + 1))
                    nc.tensor.matmul(ps_o[:1, qs], wslice(_WC_OW2, 1, 0, H),
                                     rl[:, qs], start=True, stop=True)
                pa = sc.tile([1, NA], F32, tag="pa")
                nc.vector.tensor_scalar(out=pa[:], in0=ps_o[:1, :NA],
                                        scalar1=wslice(_WC_OB2, 1, 0, 1),
                                        scalar2=None, op0=OP.add)
                en = sc.tile([1, NM], F32, tag="en")
                nc.vector.tensor_reduce(
                    out=en[:], in_=pa[:].rearrange("p (m i) -> p m i", i=APM),
                    axis=AX.X, op=OP.add)
                nc.sync.dma_start(out=out_d[:].unsqueeze(0), in_=en[:])

    nc.compile()
    return nc


def _filter_exact(l, dv, mlp_w1, mlp_b1, mlp_w2, mlp_b2):
    offset = np.linspace(0.0, CUTOFF, NG)
    coeff = -0.5 / (offset[1] - offset[0]) ** 2
    ea = np.exp(coeff * (dv[:, None] - offset[None, :]) ** 2)
    pre = ea @ mlp_w1[l] + mlp_b1[l]
    W = (np.logaddexp(0, pre) - LOG2) @ mlp_w2[l] + mlp_b2[l]
    cc = 0.5 * (np.cos(dv * np.pi / CUTOFF) + 1.0)
    return W * cc[:, None]


def _prep_inputs(z, pos, ptr, emb, mlp_w1, mlp_b1, mlp_w2, mlp_b2,
                 lin1_w, lin2_w, lin2_b, lin_w, lin_b,
                 out_w1, out_b1, out_w2, out_b2):
    z = np.asarray(z)
    pos = np.ascontiguousarray(np.asarray(pos, dtype=np.float32))
    ptr = np.asarray(ptr)
    assert pos.shape == (N, 3)
    expect = np.arange(0, N + APM, APM)
    assert np.array_equal(ptr.astype(np.int64), expect), "non-uniform molecules unsupported"

    emb = np.asarray(emb, dtype=np.float32)
    mlp_w1 = np.asarray(mlp_w1, dtype=np.float64)
    mlp_b1 = np.asarray(mlp_b1, dtype=np.float64)
    mlp_w2 = np.asarray(mlp_w2, dtype=np.float64)
    mlp_b2 = np.asarray(mlp_b2, dtype=np.float64)
    lin_w_f = np.asarray(lin_w, dtype=np.float32)
    out_w2_f = np.asarray(out_w2, dtype=np.float32)

    # least-squares tent-table fit per layer on a fine grid, last node pinned 0
    fine = np.linspace(0.0, CUTOFF, 4096)
    Cb = np.maximum(0.0, 1.0 - np.abs(fine[:, None] / TH - np.arange(T)[None, :]))
    CbL = Cb[:, :T - 1]
    wpack = np.zeros((128, WC), dtype=np.float32)
    for l in range(L):
        F = _filter_exact(l, fine, mlp_w1, mlp_b1, mlp_w2, mlp_b2)
        TBL, *_ = np.linalg.lstsq(CbL, F, rcond=None)
        TBL = np.vstack([TBL, np.zeros((1, FEAT))]).astype(np.float32)
        if OPTS["poolred"]:
            TBL = TBL * float(APM)
        wpack[0:T, _WC_TBL + l * FEAT:_WC_TBL + (l + 1) * FEAT] = TBL
        wpack[T:2 * T, _WC_TBL + l * FEAT:_WC_TBL + (l + 1) * FEAT] = TBL
    wpack[:FEAT, _WC_L1W:_WC_L1W + L * FEAT] = \
        np.asarray(lin1_w, np.float32).transpose(1, 0, 2).reshape(FEAT, L * FEAT)
    wpack[:FEAT, _WC_L2W:_WC_L2W + L * FEAT] = \
        np.asarray(lin2_w, np.float32).transpose(1, 0, 2).reshape(FEAT, L * FEAT)
    wpack[:FEAT, _WC_LW:_WC_LW + L * FEAT] = \
        lin_w_f.transpose(1, 0, 2).reshape(FEAT, L * FEAT)
    wpack[:FEAT, _WC_L2B:_WC_L2B + L] = np.asarray(lin2_b, np.float32).T
    wpack[:FEAT, _WC_LBP:_WC_LBP + L] = np.asarray(lin_b, np.float32).T
    wpack[:FEAT, _WC_OW1:_WC_OW1 + H] = np.asarray(out_w1, np.float32)
    wpack[:H, _WC_OW2] = out_w2_f.reshape(H)
    wpack[:H, _WC_OB1] = np.asarray(out_b1, np.float32)
    wpack[0, _WC_OB2] = float(np.asarray(out_b2, np.float32).reshape(()))
    for p in range(128):
        wpack[p, _WC_TV] = float(p % T) - 1.0
        wpack[p, _WC_D36 + (p % APM)] = 36.0

    in_maps = []
    for c in range(NCORES):
        sl = slice(NA * c, NA * (c + 1))
        h0 = emb[np.asarray(z[sl], dtype=np.int64)].T
        in_maps.append({
            "pos": pos[sl].copy(),
            "h0": np.ascontiguousarray(h0, dtype=np.float32),
            "wpack": wpack,
        })
    return in_maps


def kernel(**inputs) -> np.ndarray:
    from concourse.bass_utils import run_bass_kernel_spmd
    global _COMPILED
    if _COMPILED is None:
        _COMPILED = _build(1)
    nc = _COMPILED
    in_maps = _prep_inputs(**inputs)
    res = run_bass_kernel_spmd(nc, in_maps, list(range(NCORES)))
    out = np.concatenate([res.results[c]["energy"] for c in range(NCORES)])
    return out.astype(np.float32)


if __name__ == "__main__":
    _build(1)
    print("built ok")
